# revision 10
# baseline (speedup 1.0000x reference)
"""MetaPathGNN kernel for 8 Trainium2 NeuronCores.

Computation (h_b/conv0/edge_ab/x_b are dead code in the reference):
    msg  = x_a[edge_ba[1]]                      # [E, H] gather
    aggr = segment_sum(msg, edge_ba[0], N)      # [N, H]
    h_a  = relu(aggr @ wl1.T + x_a @ (w01+w11).T + (bl1+b01+b11))
    out  = h_a @ out_w.T + out_b

Sharding: destination nodes split into 8 contiguous shards of 6250; each core
gathers source rows for its own edges from a full x_a replica (no
collectives), aggregates via one-hot matmuls into PSUM, applies the linears
feature-major, writes its outT stripe; host reassembles.

Cost-model structure this kernel is built around (measured):
  - SWDGE gather transfers serialize at ~0.833 ns/row (256B f16 rows) and are
    the wall; everything else must hide under them.
  - HWDGE (regular dma_start) traffic runs fully in parallel with the SWDGE
    stream (~330 GB/s), so the one-hot S matrices are PRECOMPUTED ON HOST,
    stored fp8 (exact for 0/1), and streamed from HBM instead of being built
    on DVE (matmul accepts mixed f16 lhsT x fp8 rhs).
  - Each dma_start costs ~0.5-0.8us of SP engine time regardless of size, so
    loads are batched coarsely (whole idx stream, whole xaT, few S batches).

Aggregation: per 512-dest PSUM group, windows of <=128 dests are chosen by a
small DP to minimize the shared chunk budget sum_(w,p) max_core
ceil(edges/128) (the SPMD program is shared, so budgets are maxed across
cores; pad chunks gather row 0 with an all-zero S column block). Window
matmuls: out = msg.T @ S accumulated bank-wide (start=True pending-zeroes the
bank; every window is touched by >=1 matmul).

dma_gather indices are int16, so sources are gathered in two passes
(src < 32768 from base 0, src >= 32768 from base 32768).
"""

import numpy as np

P = 8
N = 50000
E = 500000
H = 128
NSH = N // P          # 6250 destinations per core
GROUP = 512           # PSUM bank width in fp32 columns
NG = (NSH + GROUP - 1) // GROUP   # 13
NCOL = NG * GROUP     # 6656
SPLIT = 32768         # int16-index limit for dma_gather
WMAX = 128            # max window width (matmul stationary free dim)
GCAP = 24             # gather batch cap, chunks
SCAP = 4096           # S-stream batch cap, columns


def _dp_windows(cc, nreal):
    """Choose window boundaries for one group.

    cc: [P, 2, nreal+1] per-core per-pass cumulative edge counts over local
    dest positions. Returns list of (woff, width, bA, bB) tiling [0, nreal),
    minimizing total chunk budget (ties: fewer matmul columns).
    """
    INF = 1 << 40
    dp = np.full(nreal + 1, INF, np.int64)
    dp[0] = 0
    choice = np.zeros(nreal + 1, np.int64)
    for e in range(1, nreal + 1):
        w = np.arange(1, min(WMAX, e) + 1)
        s = e - w
        nA = (cc[:, 0, e:e + 1] - cc[:, 0, s]).max(axis=0)
        nB = (cc[:, 1, e:e + 1] - cc[:, 1, s]).max(axis=0)
        bA = np.maximum(-(-nA // 128), 1)
        bB = -(-nB // 128)
        cost = dp[s] + (bA + bB) * 100000 + (bA + bB) * w
        i = int(np.argmin(cost))
        dp[e] = cost[i]
        choice[e] = w[i]
    wins = []
    e = nreal
    while e > 0:
        w = int(choice[e])
        s = e - w
        nA = int((cc[:, 0, e] - cc[:, 0, s]).max())
        nB = int((cc[:, 1, e] - cc[:, 1, s]).max())
        wins.append((s, w, max(-(-nA // 128), 1), -(-nB // 128)))
        e = s
    return wins[::-1]


def _pack_edges(dst, src):
    """Window selection, shared slot schedule, and per-core stream packing.

    Returns (slots, CA, CB, SCOLS, per_core).
    slots: list of dicts with p, g, woff, width, scol, chunk (per-pass chunk
    index), first/last (of PSUM group) in processing order.
    per_core[c]: idxA/idxB int16 streams and S fp8 [128, SCOLS].
    """
    core = dst // NSH
    dl = dst - core * NSH
    g_of = dl // GROUP
    lo = dl - g_of * GROUP
    pss = (src >= SPLIT).astype(np.int64)

    # per-group per-core per-pass cumulative counts -> DP windows
    group_wins = []
    for g in range(NG):
        nreal = min(GROUP, NSH - g * GROUP)
        cc = np.zeros((P, 2, nreal + 1), np.int64)
        m = g_of == g
        for c in range(P):
            for p in range(2):
                mm = m & (core == c) & (pss == p)
                cnt = np.bincount(lo[mm], minlength=nreal)
                cc[c, p, 1:] = np.cumsum(cnt)
        group_wins.append(_dp_windows(cc, nreal))

    # process heavy groups first so the tail chain is light
    gweight = [sum(bA + bB for _, _, bA, bB in group_wins[g]) for g in range(NG)]
    group_order = sorted(range(NG), key=lambda g: -gweight[g])

    slots = []
    ca = cb = scols = 0
    for g in group_order:
        first = len(slots)
        for woff, w, bA, bB in group_wins[g]:
            for p, b in ((0, bA), (1, bB)):
                for _ in range(b):
                    slots.append(dict(p=p, g=g, woff=woff, width=w, scol=scols,
                                      chunk=(ca if p == 0 else cb),
                                      first=False, last=False))
                    scols += w
                    if p == 0:
                        ca += 1
                    else:
                        cb += 1
        slots[first]["first"] = True
        slots[-1]["last"] = True
    CA, CB, SCOLS = ca, cb, scols

    # per-core packing: bucket edges by (g, window, pass), chunk, emit streams
    import ml_dtypes
    win_id = np.zeros(len(dl), np.int64)
    win_off = np.zeros(len(dl), np.int64)
    wid_base = {}
    wb = 0
    for g in range(NG):
        bounds = np.array([woff for woff, _, _, _ in group_wins[g]] + [GROUP])  # edges all < nreal
        m = g_of == g
        wi = np.searchsorted(bounds, lo[m], side="right") - 1
        win_id[m] = wb + wi
        win_off[m] = lo[m] - bounds[wi]
        wid_base[g] = wb
        wb += len(group_wins[g])

    per_core = []
    for c in range(P):
        m = core == c
        key = (win_id[m] * 2 + pss[m])
        order = np.lexsort((lo[m], key))
        kk = key[order]
        srcs = src[m][order]
        woffs = win_off[m][order]
        bstart = {}
        uk, first_idx = np.unique(kk, return_index=True)
        for u, fi in zip(uk, first_idx):
            bstart[int(u)] = int(fi)
        bcount = {int(u): int(n) for u, n in zip(uk, np.bincount(np.searchsorted(uk, kk)))}

        idxs = [np.zeros(CA * 128, np.int64), np.zeros(CB * 128, np.int64)]
        s_rows = []
        s_cols = []
        taken = {}
        for sl in slots:
            kq = (wid_base[sl["g"]] + _win_index(group_wins[sl["g"]], sl["woff"])) * 2 + sl["p"]
            t = taken.get(kq, 0)
            n = min(128, max(0, bcount.get(kq, 0) - t))
            if n:
                b0 = bstart[kq] + t
                rows = np.arange(n)
                ss = srcs[b0:b0 + n] - (SPLIT if sl["p"] else 0)
                idxs[sl["p"]][sl["chunk"] * 128:sl["chunk"] * 128 + n] = ss
                s_rows.append(rows)
                s_cols.append(sl["scol"] + woffs[b0:b0 + n])
                taken[kq] = t + n
        S = np.zeros((128, SCOLS), ml_dtypes.float8_e4m3fn)
        if s_rows:
            S[np.concatenate(s_rows), np.concatenate(s_cols)] = 1.0
        per_core.append({
            "idxA": idxs[0].astype(np.int16),
            "idxB": idxs[1].astype(np.int16),
            "S": S,
        })
    return slots, CA, CB, SCOLS, per_core


def _win_index(wins, woff):
    for i, (o, _, _, _) in enumerate(wins):
        if o == woff:
            return i
    raise KeyError(woff)


def _wrap_idx(idx):
    """dma_gather index layout: element i at [i % 16, i // 16], tiled to 128
    partitions."""
    w = np.ascontiguousarray(idx.reshape(-1, 16).T)  # [16, L/16]
    return np.tile(w, (8, 1))


def _make_batches(total, ramp=(4, 8, 16), cap=GCAP, tailmax=8):
    """Batch sizes: small ramp-in, cap-sized middle, small final batch."""
    sizes = []
    left = total
    for r in ramp:
        if left <= 0:
            break
        s = min(r, left)
        sizes.append(s)
        left -= s
    while left > 0:
        s = min(cap, left)
        sizes.append(s)
        left -= s
    if len(sizes) > 1 and sizes[-1] > tailmax:
        sizes[-1] -= tailmax
        sizes.append(tailmax)
    out = []
    st = 0
    for s in sizes:
        out.append((st, s))
        st += s
    return out


def _make_sbatches(slots, first=1024, cap=SCAP):
    """S-stream batches cut at slot boundaries: list of (startcol, ncols)."""
    bounds = [0]
    lim = first
    for sl in slots:
        end = sl["scol"] + sl["width"]
        if end - bounds[-1] > lim:
            bounds.append(sl["scol"])
            lim = cap
    total = slots[-1]["scol"] + slots[-1]["width"]
    bounds.append(total)
    return [(bounds[i], bounds[i + 1] - bounds[i]) for i in range(len(bounds) - 1)
            if bounds[i + 1] > bounds[i]]


def _build_program(slots, CA, CB, SCOLS):
    import concourse.bacc as bacc
    import concourse.tile as tile
    import concourse.mybir as mybir

    F32 = mybir.dt.float32
    F16 = mybir.dt.float16
    FP8 = mybir.dt.float8e4
    I16 = mybir.dt.int16

    nc = bacc.Bacc("TRN2", num_swdge_queues=1, dynamic_dma_scratch_size=98304)
    xa_d = nc.dram_tensor("xa", [N, H], F16, kind="ExternalInput")
    xaT_d = nc.dram_tensor("xaT", [H, NCOL], F16, kind="ExternalInput")
    idx_d = nc.dram_tensor("idx", [128, (CA + CB) * 8], I16, kind="ExternalInput")
    s_d = nc.dram_tensor("smat", [128, SCOLS], FP8, kind="ExternalInput")
    wagg_d = nc.dram_tensor("wagg", [H, H], F16, kind="ExternalInput")
    wx_d = nc.dram_tensor("wx", [H, H], F16, kind="ExternalInput")
    wo_d = nc.dram_tensor("wo", [H, H], F16, kind="ExternalInput")
    bh_d = nc.dram_tensor("bh", [H, 1], F32, kind="ExternalInput")
    outT_d = nc.dram_tensor("outT", [H, NCOL], F16, kind="ExternalOutput")

    batches = [_make_batches(CA), _make_batches(CB)]
    gbmax = max(n for bl in batches for _, n in bl)
    sbatches = _make_sbatches(slots)
    scmax = max(n for _, n in sbatches)
    base = [(0, SPLIT), (SPLIT, N)]
    idx_off = [0, CA * 8]   # column offset of each pass in the idx tile

    relu = mybir.ActivationFunctionType.Relu
    copyf = mybir.ActivationFunctionType.Copy

    with tile.TileContext(nc) as tc:
        with (
            tc.tile_pool(name="const", bufs=1) as constp,
            tc.tile_pool(name="gath", bufs=3) as gathp,
            tc.tile_pool(name="spool", bufs=2) as spool,
            tc.tile_pool(name="post", bufs=2) as postp,
            tc.tile_pool(name="ps", bufs=2, space="PSUM") as psump,
        ):
            # whole idx stream resident; head slice first so batch 0 starts fast
            idx_t = constp.tile([128, (CA + CB) * 8], I16, tag="idx")
            headA = min(CA, batches[0][0][1])
            headB = min(CB, batches[1][0][1])
            nc.sync.dma_start(idx_t[:, :headA * 8], idx_d[:, :headA * 8])
            nc.sync.dma_start(idx_t[:, CA * 8:CA * 8 + headB * 8],
                              idx_d[:, CA * 8:CA * 8 + headB * 8])
            if CA > headA:
                nc.sync.dma_start(idx_t[:, headA * 8:CA * 8],
                                  idx_d[:, headA * 8:CA * 8])
            if CB > headB:
                nc.sync.dma_start(idx_t[:, CA * 8 + headB * 8:],
                                  idx_d[:, CA * 8 + headB * 8:])

            wagg_t = constp.tile([H, H], F16, tag="wagg")
            wx_t = constp.tile([H, H], F16, tag="wx")
            wo_t = constp.tile([H, H], F16, tag="wo")
            bh_t = constp.tile([H, 1], F32, tag="bh")
            xaT_t = constp.tile([128, NCOL], F16, tag="xaT")
            for t, dd in ((wagg_t, wagg_d), (wx_t, wx_d), (wo_t, wo_d),
                          (bh_t, bh_d), (xaT_t, xaT_d)):
                nc.sync.dma_start(t[:], dd[:])

            # streaming state
            cur_g = [None, None]      # current gather tile per pass
            gstart = [0, 0]
            gpos = [0, 0]             # next batch index per pass
            consumed = [0, 0]
            cur_s = [None]            # current S tile
            sstart = [0]
            spos = [0]
            qrr = [0]

            def lhsT_for(p, c):
                if cur_g[p] is None or c >= gstart[p] + cur_g[p].shape[1]:
                    st, nch = batches[p][gpos[p]]
                    assert st == c, (p, c, st)
                    gpos[p] += 1
                    gstart[p] = st
                    t = gathp.tile([128, gbmax, H], F16, tag=f"g{p}")
                    t = t[:, :nch, :]
                    lo, hi = base[p]
                    nc.gpsimd.dma_gather(
                        t[:], xa_d[lo:hi, :],
                        idx_t[:, idx_off[p] + st * 8: idx_off[p] + (st + nch) * 8],
                        nch * 128, nch * 128, H,
                        single_packet=False, queue_num=0,
                    )
                    cur_g[p] = t
                return cur_g[p][:, c - gstart[p], :]

            def s_for(scol, w):
                if cur_s[0] is None or scol >= sstart[0] + cur_s[0].shape[1]:
                    st, ncols = sbatches[spos[0]]
                    assert st == scol, (scol, st)
                    spos[0] += 1
                    sstart[0] = st
                    t = spool.tile([128, scmax], FP8, tag="s")
                    t = t[:, :ncols]
                    nc.sync.dma_start(t[:], s_d[:, st:st + ncols])
                    cur_s[0] = t
                return cur_s[0][:, scol - sstart[0]: scol - sstart[0] + w]

            aggr_ps = None
            for sl in slots:
                if sl["first"]:
                    aggr_ps = psump.tile([128, GROUP], F32, tag="aggr")
                p = sl["p"]
                lhsT = lhsT_for(p, consumed[p])
                consumed[p] += 1
                rhs = s_for(sl["scol"], sl["width"])
                nc.tensor.matmul(
                    aggr_ps[:, sl["woff"]:sl["woff"] + sl["width"]],
                    lhsT, rhs, start=sl["first"], stop=sl["last"],
                )
                if sl["last"]:
                    g = sl["g"]
                    nr = min(GROUP, NSH - g * GROUP)
                    aggr_sb = postp.tile([128, GROUP], F16, tag="aggr_sb")
                    nc.vector.tensor_scalar_mul(aggr_sb[:, :nr], aggr_ps[:, :nr], 1.0)
                    z_ps = psump.tile([128, GROUP], F32, tag="z")
                    nc.tensor.matmul(z_ps[:, :nr], wagg_t[:], aggr_sb[:, :nr],
                                     start=True, stop=False)
                    nc.tensor.matmul(z_ps[:, :nr], wx_t[:],
                                     xaT_t[:, g * GROUP:g * GROUP + nr],
                                     start=False, stop=True)
                    h_sb = postp.tile([128, GROUP], F16, tag="h")
                    nc.vector.tensor_scalar(
                        out=h_sb[:, :nr], in0=z_ps[:, :nr],
                        scalar1=bh_t[:, 0:1], scalar2=0.0,
                        op0=mybir.AluOpType.add, op1=mybir.AluOpType.max)
                    o_ps = psump.tile([128, GROUP], F32, tag="o")
                    nc.tensor.matmul(o_ps[:, :nr], wo_t[:], h_sb[:, :nr],
                                     start=True, stop=True)
                    o_sb = postp.tile([128, GROUP], F16, tag="osb")
                    nc.scalar.activation(o_sb[:, :nr], o_ps[:, :nr], copyf)
                    nc.sync.dma_start(outT_d[:, g * GROUP:g * GROUP + nr],
                                      o_sb[:, :nr])

    nc.compile()
    return nc


def prepare(inputs):
    """Host-side packing: returns (nc, in_maps)."""
    x_a = np.ascontiguousarray(np.asarray(inputs["x_a"], dtype=np.float32))
    eb = np.asarray(inputs["edge_ba"])
    dst = eb[0].astype(np.int64)
    src = eb[1].astype(np.int64)

    wagg = np.ascontiguousarray(
        np.asarray(inputs["conv1_wl_w"], np.float32).T.astype(np.float16))
    wx = np.ascontiguousarray(
        (np.asarray(inputs["conv1_w0_w"], np.float32)
         + np.asarray(inputs["conv1_w1_w"], np.float32)).T.astype(np.float16))
    bh = (np.asarray(inputs["conv1_wl_b"], np.float32)
          + np.asarray(inputs["conv1_w0_b"], np.float32)
          + np.asarray(inputs["conv1_w1_b"], np.float32)).reshape(H, 1)
    wo = np.ascontiguousarray(np.asarray(inputs["out_w"], np.float32).T.astype(np.float16))
    xa16 = x_a.astype(np.float16)

    slots, CA, CB, SCOLS, per_core = _pack_edges(dst, src)
    nc = _build_program(slots, CA, CB, SCOLS)

    in_maps = []
    for c in range(P):
        xaT = np.zeros((H, NCOL), np.float16)
        xaT[:, :NSH] = x_a[c * NSH:(c + 1) * NSH].T.astype(np.float16)
        a = per_core[c]
        idx = np.concatenate([
            _wrap_idx(a["idxA"]), _wrap_idx(a["idxB"])], axis=1)
        in_maps.append({
            "xa": xa16,
            "xaT": xaT,
            "idx": idx,
            "smat": a["S"],
            "wagg": wagg, "wx": wx, "wo": wo, "bh": bh,
        })
    return nc, in_maps, np.asarray(inputs["out_b"], np.float32)


def assemble(results, out_b):
    out = np.empty((N, H), np.float32)
    for c in range(P):
        out[c * NSH:(c + 1) * NSH] = results[c]["outT"][:, :NSH].T.astype(np.float32)
    out += out_b[None, :]
    return out


def kernel(**inputs):
    from concourse.bass_utils import run_bass_kernel_spmd

    nc, in_maps, out_b = prepare(inputs)
    r = run_bass_kernel_spmd(nc, in_maps, list(range(P)))
    return assemble(r.results, out_b)


# revision 11
# speedup vs baseline: 1.0604x; 1.0604x over previous
"""MetaPathGNN kernel for 8 Trainium2 NeuronCores.

Computation (h_b/conv0/edge_ab/x_b are dead code in the reference):
    msg  = x_a[edge_ba[1]]                      # [E, H] gather
    aggr = segment_sum(msg, edge_ba[0], N)      # [N, H]
    h_a  = relu(aggr @ wl1.T + x_a @ (w01+w11).T + (bl1+b01+b11))
    out  = h_a @ out_w.T + out_b

Sharding: destination nodes split into 8 contiguous shards of 6250; each core
gathers source rows for its own edges from a full x_a replica (no
collectives), aggregates via one-hot matmuls into PSUM, applies the linears
feature-major, writes its outT stripe; host reassembles.

Cost-model structure this kernel is built around (measured):
  - SWDGE gather transfers serialize at ~0.833 ns/row (256B f16 rows) and are
    the wall; everything else must hide under them.
  - HWDGE (regular dma_start) traffic runs fully in parallel with the SWDGE
    stream (~330 GB/s), so the one-hot S matrices are PRECOMPUTED ON HOST,
    stored fp8 (exact for 0/1), and streamed from HBM instead of being built
    on DVE (matmul accepts mixed f16 lhsT x fp8 rhs).
  - Each dma_start costs ~0.5-0.8us of SP engine time regardless of size, so
    loads are batched coarsely (whole idx stream, whole xaT, few S batches).

Aggregation: per 512-dest PSUM group, windows of <=128 dests are chosen by a
small DP to minimize the shared chunk budget sum_(w,p) max_core
ceil(edges/128) (the SPMD program is shared, so budgets are maxed across
cores; pad chunks gather row 0 with an all-zero S column block). Window
matmuls: out = msg.T @ S accumulated bank-wide (start=True pending-zeroes the
bank; every window is touched by >=1 matmul).

dma_gather indices are int16, so sources are gathered in two passes
(src < 32768 from base 0, src >= 32768 from base 32768).
"""

import numpy as np

P = 8
N = 50000
E = 500000
H = 128
NSH = N // P          # 6250 destinations per core
GROUP = 512           # PSUM bank width in fp32 columns
NG = (NSH + GROUP - 1) // GROUP   # 13
NCOL = NG * GROUP     # 6656
SPLIT = 32768         # int16-index limit for dma_gather
WMAX = 128            # max window width (matmul stationary free dim)
GCAP = 48             # gather batch cap, chunks
SCAP = 4096           # S-stream batch cap, columns


def _dp_windows(cc, nreal):
    """Choose window boundaries for one group.

    cc: [P, 2, nreal+1] per-core per-pass cumulative edge counts over local
    dest positions. Returns list of (woff, width, bA, bB) tiling [0, nreal),
    minimizing total chunk budget (ties: fewer matmul columns).
    """
    INF = 1 << 40
    dp = np.full(nreal + 1, INF, np.int64)
    dp[0] = 0
    choice = np.zeros(nreal + 1, np.int64)
    for e in range(1, nreal + 1):
        w = np.arange(1, min(WMAX, e) + 1)
        s = e - w
        nA = (cc[:, 0, e:e + 1] - cc[:, 0, s]).max(axis=0)
        nB = (cc[:, 1, e:e + 1] - cc[:, 1, s]).max(axis=0)
        bA = np.maximum(-(-nA // 128), 1)
        bB = -(-nB // 128)
        cost = dp[s] + (bA + bB) * 100000 + (bA + bB) * w
        i = int(np.argmin(cost))
        dp[e] = cost[i]
        choice[e] = w[i]
    wins = []
    e = nreal
    while e > 0:
        w = int(choice[e])
        s = e - w
        nA = int((cc[:, 0, e] - cc[:, 0, s]).max())
        nB = int((cc[:, 1, e] - cc[:, 1, s]).max())
        wins.append((s, w, max(-(-nA // 128), 1), -(-nB // 128)))
        e = s
    return wins[::-1]


def _pack_edges(dst, src):
    """Window selection, shared slot schedule, and per-core stream packing.

    Returns (slots, CA, CB, SCOLS, per_core).
    slots: list of dicts with p, g, woff, width, scol, chunk (per-pass chunk
    index), first/last (of PSUM group) in processing order.
    per_core[c]: idxA/idxB int16 streams and S fp8 [128, SCOLS].
    """
    core = dst // NSH
    dl = dst - core * NSH
    g_of = dl // GROUP
    lo = dl - g_of * GROUP
    pss = (src >= SPLIT).astype(np.int64)

    # per-group per-core per-pass cumulative counts -> DP windows
    group_wins = []
    for g in range(NG):
        nreal = min(GROUP, NSH - g * GROUP)
        cc = np.zeros((P, 2, nreal + 1), np.int64)
        m = g_of == g
        for c in range(P):
            for p in range(2):
                mm = m & (core == c) & (pss == p)
                cnt = np.bincount(lo[mm], minlength=nreal)
                cc[c, p, 1:] = np.cumsum(cnt)
        group_wins.append(_dp_windows(cc, nreal))

    # process heavy groups first so the tail chain is light
    gweight = [sum(bA + bB for _, _, bA, bB in group_wins[g]) for g in range(NG)]
    group_order = sorted(range(NG), key=lambda g: -gweight[g])

    slots = []
    ca = cb = scols = 0
    for g in group_order:
        first = len(slots)
        for woff, w, bA, bB in group_wins[g]:
            for p, b in ((0, bA), (1, bB)):
                for _ in range(b):
                    slots.append(dict(p=p, g=g, woff=woff, width=w, scol=scols,
                                      chunk=(ca if p == 0 else cb),
                                      first=False, last=False))
                    scols += w
                    if p == 0:
                        ca += 1
                    else:
                        cb += 1
        slots[first]["first"] = True
        slots[-1]["last"] = True
    CA, CB, SCOLS = ca, cb, scols

    # per-core packing: bucket edges by (g, window, pass), chunk, emit streams
    import ml_dtypes
    win_id = np.zeros(len(dl), np.int64)
    win_off = np.zeros(len(dl), np.int64)
    wid_base = {}
    wb = 0
    for g in range(NG):
        bounds = np.array([woff for woff, _, _, _ in group_wins[g]] + [GROUP])  # edges all < nreal
        m = g_of == g
        wi = np.searchsorted(bounds, lo[m], side="right") - 1
        win_id[m] = wb + wi
        win_off[m] = lo[m] - bounds[wi]
        wid_base[g] = wb
        wb += len(group_wins[g])

    per_core = []
    for c in range(P):
        m = core == c
        key = (win_id[m] * 2 + pss[m])
        order = np.lexsort((lo[m], key))
        kk = key[order]
        srcs = src[m][order]
        woffs = win_off[m][order]
        bstart = {}
        uk, first_idx = np.unique(kk, return_index=True)
        for u, fi in zip(uk, first_idx):
            bstart[int(u)] = int(fi)
        bcount = {int(u): int(n) for u, n in zip(uk, np.bincount(np.searchsorted(uk, kk)))}

        idxs = [np.zeros(CA * 128, np.int64), np.zeros(CB * 128, np.int64)]
        s_rows = []
        s_cols = []
        taken = {}
        for sl in slots:
            kq = (wid_base[sl["g"]] + _win_index(group_wins[sl["g"]], sl["woff"])) * 2 + sl["p"]
            t = taken.get(kq, 0)
            n = min(128, max(0, bcount.get(kq, 0) - t))
            if n:
                b0 = bstart[kq] + t
                rows = np.arange(n)
                ss = srcs[b0:b0 + n] - (SPLIT if sl["p"] else 0)
                idxs[sl["p"]][sl["chunk"] * 128:sl["chunk"] * 128 + n] = ss
                s_rows.append(rows)
                s_cols.append(sl["scol"] + woffs[b0:b0 + n])
                taken[kq] = t + n
        S = np.zeros((128, SCOLS), ml_dtypes.float8_e4m3fn)
        if s_rows:
            S[np.concatenate(s_rows), np.concatenate(s_cols)] = 1.0
        per_core.append({
            "idxA": idxs[0].astype(np.int16),
            "idxB": idxs[1].astype(np.int16),
            "S": S,
        })
    return slots, CA, CB, SCOLS, per_core


def _win_index(wins, woff):
    for i, (o, _, _, _) in enumerate(wins):
        if o == woff:
            return i
    raise KeyError(woff)


def _wrap_idx(idx):
    """dma_gather index layout: element i at [i % 16, i // 16], tiled to 128
    partitions."""
    w = np.ascontiguousarray(idx.reshape(-1, 16).T)  # [16, L/16]
    return np.tile(w, (8, 1))


def _make_batches(total, ramp=(8, 16, 32), cap=GCAP, tailmax=16):
    """Batch sizes: small ramp-in, cap-sized middle, small final batch."""
    sizes = []
    left = total
    for r in ramp:
        if left <= 0:
            break
        s = min(r, left)
        sizes.append(s)
        left -= s
    while left > 0:
        s = min(cap, left)
        sizes.append(s)
        left -= s
    if len(sizes) > 1 and sizes[-1] > tailmax:
        sizes[-1] -= tailmax
        sizes.append(tailmax)
    out = []
    st = 0
    for s in sizes:
        out.append((st, s))
        st += s
    return out


def _make_sbatches(slots, first=1024, cap=SCAP):
    """S-stream batches cut at slot boundaries: list of (startcol, ncols)."""
    bounds = [0]
    lim = first
    for sl in slots:
        end = sl["scol"] + sl["width"]
        if end - bounds[-1] > lim:
            bounds.append(sl["scol"])
            lim = cap
    total = slots[-1]["scol"] + slots[-1]["width"]
    bounds.append(total)
    return [(bounds[i], bounds[i + 1] - bounds[i]) for i in range(len(bounds) - 1)
            if bounds[i + 1] > bounds[i]]


def _build_program(slots, CA, CB, SCOLS):
    import concourse.bacc as bacc
    import concourse.tile as tile
    import concourse.mybir as mybir

    F32 = mybir.dt.float32
    F16 = mybir.dt.float16
    FP8 = mybir.dt.float8e4
    I16 = mybir.dt.int16

    nc = bacc.Bacc("TRN2", num_swdge_queues=4, dynamic_dma_scratch_size=98304)
    xa_d = nc.dram_tensor("xa", [N, H], F16, kind="ExternalInput")
    xaT_d = nc.dram_tensor("xaT", [H, NCOL], F16, kind="ExternalInput")
    idx_d = nc.dram_tensor("idx", [128, (CA + CB) * 8], I16, kind="ExternalInput")
    s_d = nc.dram_tensor("smat", [128, SCOLS], FP8, kind="ExternalInput")
    wagg_d = nc.dram_tensor("wagg", [H, H], F16, kind="ExternalInput")
    wx_d = nc.dram_tensor("wx", [H, H], F16, kind="ExternalInput")
    wo_d = nc.dram_tensor("wo", [H, H], F16, kind="ExternalInput")
    bh_d = nc.dram_tensor("bh", [H, 1], F32, kind="ExternalInput")
    outT_d = nc.dram_tensor("outT", [H, NCOL], F16, kind="ExternalOutput")

    batches = [_make_batches(CA), _make_batches(CB)]
    gbmax = max(n for bl in batches for _, n in bl)
    sbatches = _make_sbatches(slots)
    scmax = max(n for _, n in sbatches)
    base = [(0, SPLIT), (SPLIT, N)]
    idx_off = [0, CA * 8]   # column offset of each pass in the idx tile

    relu = mybir.ActivationFunctionType.Relu
    copyf = mybir.ActivationFunctionType.Copy

    with tile.TileContext(nc) as tc:
        with (
            tc.tile_pool(name="const", bufs=1) as constp,
            tc.tile_pool(name="gath", bufs=3) as gathp,
            tc.tile_pool(name="spool", bufs=2) as spool,
            tc.tile_pool(name="post", bufs=2) as postp,
            tc.tile_pool(name="ps", bufs=2, space="PSUM") as psump,
        ):
            # whole idx stream resident; head slice first so batch 0 starts fast
            idx_t = constp.tile([128, (CA + CB) * 8], I16, tag="idx")
            headA = min(CA, batches[0][0][1])
            headB = min(CB, batches[1][0][1])
            nc.sync.dma_start(idx_t[:, :headA * 8], idx_d[:, :headA * 8])
            nc.sync.dma_start(idx_t[:, CA * 8:CA * 8 + headB * 8],
                              idx_d[:, CA * 8:CA * 8 + headB * 8])
            if CA > headA:
                nc.sync.dma_start(idx_t[:, headA * 8:CA * 8],
                                  idx_d[:, headA * 8:CA * 8])
            if CB > headB:
                nc.sync.dma_start(idx_t[:, CA * 8 + headB * 8:],
                                  idx_d[:, CA * 8 + headB * 8:])

            wagg_t = constp.tile([H, H], F16, tag="wagg")
            wx_t = constp.tile([H, H], F16, tag="wx")
            wo_t = constp.tile([H, H], F16, tag="wo")
            bh_t = constp.tile([H, 1], F32, tag="bh")
            xaT_t = constp.tile([128, NCOL], F16, tag="xaT")
            for t, dd in ((wagg_t, wagg_d), (wx_t, wx_d), (wo_t, wo_d),
                          (bh_t, bh_d), (xaT_t, xaT_d)):
                nc.sync.dma_start(t[:], dd[:])

            # streaming state
            cur_g = [None, None]      # current gather tile per pass
            gstart = [0, 0]
            gpos = [0, 0]             # next batch index per pass
            consumed = [0, 0]
            cur_s = [None]            # current S tile
            sstart = [0]
            spos = [0]
            qrr = [0]

            def lhsT_for(p, c):
                if cur_g[p] is None or c >= gstart[p] + cur_g[p].shape[1]:
                    st, nch = batches[p][gpos[p]]
                    assert st == c, (p, c, st)
                    gpos[p] += 1
                    gstart[p] = st
                    t = gathp.tile([128, gbmax, H], F16, tag=f"g{p}")
                    t = t[:, :nch, :]
                    lo, hi = base[p]
                    nc.gpsimd.dma_gather(
                        t[:], xa_d[lo:hi, :],
                        idx_t[:, idx_off[p] + st * 8: idx_off[p] + (st + nch) * 8],
                        nch * 128, nch * 128, H,
                        single_packet=False, queue_num=qrr[0] % 4,
                    )
                    qrr[0] += 1
                    cur_g[p] = t
                return cur_g[p][:, c - gstart[p], :]

            def s_for(scol, w):
                if cur_s[0] is None or scol >= sstart[0] + cur_s[0].shape[1]:
                    st, ncols = sbatches[spos[0]]
                    assert st == scol, (scol, st)
                    spos[0] += 1
                    sstart[0] = st
                    t = spool.tile([128, scmax], FP8, tag="s")
                    t = t[:, :ncols]
                    nc.sync.dma_start(t[:], s_d[:, st:st + ncols])
                    cur_s[0] = t
                return cur_s[0][:, scol - sstart[0]: scol - sstart[0] + w]

            aggr_ps = None
            for sl in slots:
                if sl["first"]:
                    aggr_ps = psump.tile([128, GROUP], F32, tag="aggr")
                p = sl["p"]
                lhsT = lhsT_for(p, consumed[p])
                consumed[p] += 1
                rhs = s_for(sl["scol"], sl["width"])
                nc.tensor.matmul(
                    aggr_ps[:, sl["woff"]:sl["woff"] + sl["width"]],
                    lhsT, rhs, start=sl["first"], stop=sl["last"],
                )
                if sl["last"]:
                    g = sl["g"]
                    nr = min(GROUP, NSH - g * GROUP)
                    aggr_sb = postp.tile([128, GROUP], F16, tag="aggr_sb")
                    nc.vector.tensor_scalar_mul(aggr_sb[:, :nr], aggr_ps[:, :nr], 1.0)
                    z_ps = psump.tile([128, GROUP], F32, tag="z")
                    nc.tensor.matmul(z_ps[:, :nr], wagg_t[:], aggr_sb[:, :nr],
                                     start=True, stop=False)
                    nc.tensor.matmul(z_ps[:, :nr], wx_t[:],
                                     xaT_t[:, g * GROUP:g * GROUP + nr],
                                     start=False, stop=True)
                    h_sb = postp.tile([128, GROUP], F16, tag="h")
                    nc.vector.tensor_scalar(
                        out=h_sb[:, :nr], in0=z_ps[:, :nr],
                        scalar1=bh_t[:, 0:1], scalar2=0.0,
                        op0=mybir.AluOpType.add, op1=mybir.AluOpType.max)
                    o_ps = psump.tile([128, GROUP], F32, tag="o")
                    nc.tensor.matmul(o_ps[:, :nr], wo_t[:], h_sb[:, :nr],
                                     start=True, stop=True)
                    o_sb = postp.tile([128, GROUP], F16, tag="osb")
                    nc.scalar.activation(o_sb[:, :nr], o_ps[:, :nr], copyf)
                    nc.sync.dma_start(outT_d[:, g * GROUP:g * GROUP + nr],
                                      o_sb[:, :nr])

    nc.compile()
    return nc


def prepare(inputs):
    """Host-side packing: returns (nc, in_maps)."""
    x_a = np.ascontiguousarray(np.asarray(inputs["x_a"], dtype=np.float32))
    eb = np.asarray(inputs["edge_ba"])
    dst = eb[0].astype(np.int64)
    src = eb[1].astype(np.int64)

    wagg = np.ascontiguousarray(
        np.asarray(inputs["conv1_wl_w"], np.float32).T.astype(np.float16))
    wx = np.ascontiguousarray(
        (np.asarray(inputs["conv1_w0_w"], np.float32)
         + np.asarray(inputs["conv1_w1_w"], np.float32)).T.astype(np.float16))
    bh = (np.asarray(inputs["conv1_wl_b"], np.float32)
          + np.asarray(inputs["conv1_w0_b"], np.float32)
          + np.asarray(inputs["conv1_w1_b"], np.float32)).reshape(H, 1)
    wo = np.ascontiguousarray(np.asarray(inputs["out_w"], np.float32).T.astype(np.float16))
    xa16 = x_a.astype(np.float16)

    slots, CA, CB, SCOLS, per_core = _pack_edges(dst, src)
    nc = _build_program(slots, CA, CB, SCOLS)

    in_maps = []
    for c in range(P):
        xaT = np.zeros((H, NCOL), np.float16)
        xaT[:, :NSH] = x_a[c * NSH:(c + 1) * NSH].T.astype(np.float16)
        a = per_core[c]
        idx = np.concatenate([
            _wrap_idx(a["idxA"]), _wrap_idx(a["idxB"])], axis=1)
        in_maps.append({
            "xa": xa16,
            "xaT": xaT,
            "idx": idx,
            "smat": a["S"],
            "wagg": wagg, "wx": wx, "wo": wo, "bh": bh,
        })
    return nc, in_maps, np.asarray(inputs["out_b"], np.float32)


def assemble(results, out_b):
    out = np.empty((N, H), np.float32)
    for c in range(P):
        out[c * NSH:(c + 1) * NSH] = results[c]["outT"][:, :NSH].T.astype(np.float32)
    out += out_b[None, :]
    return out


def kernel(**inputs):
    from concourse.bass_utils import run_bass_kernel_spmd

    nc, in_maps, out_b = prepare(inputs)
    r = run_bass_kernel_spmd(nc, in_maps, list(range(P)))
    return assemble(r.results, out_b)
